# revision 1
# baseline (speedup 1.0000x reference)
"""Trainium2 Bass kernel for nn_CRFTModule (moe_routing).

Pure data parallel over batch: 8 cores, one batch row (4096 tokens) each.

Math per core (batch b, S=4096 tokens, H=1024):
  z      = gelu(x @ W1 + b1) @ W2 + b2              (critical-path detector)
  mask   = z > logit(0.7)                            (compare in logit space)
  aw     = softmax(x[last] @ sel_w + sel_b)          (adapter selector, 4-way)
  t      = gelu(x @ Dcat + db)                       (all 4 down-projs, [S,32])
  wm     = 0.3 * mask * (sum_a aw[a] (t_a @ up_w[a] + up_b[a]))
  out    = x + wm

Device pipeline (f16 matmuls, fp32 psum accumulation, fp32 residual):
  - x loaded naturally [tok, H] (one 2MB DMA per 512-token tile), cast to
    f16 (DVE), transposed on PE (128x128 blocks) into XT [H-part, tok].
  - detector mm1 runs transposed; mm2 runs back to NATURAL orientation
    (lhsT = gelu tile, rhs = W2 column) so the mask is a per-partition
    [tok,1] scalar.
  - adapter weights (softmax, per core) are folded into the up matrix once
    (U36w = U36 * wvec), the up matmul output is natural [tok, H], and the
    final op fuses mask-mult + residual-add in one DVE scalar_tensor_tensor.
  - softmax computed with the tanh identity so the whole kernel uses one
    ACT table set (gelu_and_others).
"""
import math

import numpy as np

import concourse.bacc as bacc
import concourse.mybir as mybir
from concourse.bass import ts
from concourse.tile import TileContext
from concourse.bass_utils import run_bass_kernel_spmd

dt = mybir.dt
AF = mybir.ActivationFunctionType
ALU = mybir.AluOpType

B, S, H = 8, 4096, 1024
A_DIM, N_ADAPT = 8, 4
PD = H // 2              # 512 detector hidden dim
SCALE = 0.3
THRESH = 0.7
N_CORES = 8
T = 512                  # tokens per tile
N_TILES = S // T         # 8
KUP = N_ADAPT * A_DIM + N_ADAPT  # 36

# f32 const blob column layout: b1(4) thr(1) selw(32) db(1) selb(1) o14(4) i4(4) e36(36)
_F32_COLS = 83
# f16 const blob column layout: w1(8*512) | dcat(8*32) | w2(4) | idh(128) | onesblk(32)
_F16_COLS = 8 * PD + 8 * 32 + 4 + 128 + 32


def _build():
    nc = bacc.Bacc("TRN2", target_bir_lowering=False, debug=False)

    x = nc.declare_dram_parameter("x", [S, H], dt.float32, isOutput=False)
    fb16 = nc.declare_dram_parameter("fb16", [128, _F16_COLS], dt.float16, isOutput=False)
    fb32 = nc.declare_dram_parameter("fb32", [128, _F32_COLS], dt.float32, isOutput=False)
    u36 = nc.declare_dram_parameter("u36", [128, H], dt.float16, isOutput=False)
    out = nc.declare_dram_parameter("out", [S, H], dt.float32, isOutput=True)

    with TileContext(nc) as tc:
        with (
            tc.tile_pool(name="consts", bufs=1) as cp,
            tc.tile_pool(name="work", bufs=2) as wp,
            tc.tile_pool(name="psum", bufs=2, space="PSUM") as pp,
        ):
            # prefetch tile 0 activations (two halves) before const loads
            X0 = wp.tile([128, 4, H], dt.float32, tag="X", name="Xpre", bufs=4)
            for h in range(2):
                nc.sync.dma_start(
                    out=X0[:, 2 * h : 2 * h + 2, :],
                    in_=x[h * 256 : (h + 1) * 256, :].rearrange(
                        "(j p) h -> p j h", p=128
                    ),
                )

            # ---- constants ----
            # idh (identity) first: transposes need it before the big blob lands
            c16 = cp.tile([128, _F16_COLS], dt.float16, tag="c16")
            _ID0 = 8 * PD + 260
            nc.sync.dma_start(out=c16[:, _ID0:_F16_COLS], in_=fb16[:, _ID0:_F16_COLS])
            # w1 chunks c=0..3 first so mm1 can start before the full blob lands
            nc.sync.dma_start(out=c16[:, 0 : 4 * PD], in_=fb16[:, 0 : 4 * PD])
            nc.sync.dma_start(out=c16[:, 4 * PD : _ID0], in_=fb16[:, 4 * PD : _ID0])
            c32 = cp.tile([128, _F32_COLS], dt.float32, tag="c32")
            nc.sync.dma_start(out=c32[:], in_=fb32[:])
            u_sb = cp.tile([128, H], dt.float16, tag="u36")
            nc.sync.dma_start(out=u_sb[:], in_=u36[:])
            xlast0 = cp.tile([128, 8], dt.float32, tag="xlast")
            nc.sync.dma_start(
                out=xlast0[:],
                in_=x.rearrange("s (c p) -> p s c", p=128)[:, S - 1, :],
            )

            w1v = c16[:, 0 : 8 * PD].rearrange("p (c n) -> p c n", c=8)
            dcv = c16[:, 8 * PD : 8 * PD + 256].rearrange("p (c n) -> p c n", c=8)
            w2v = c16[:, 8 * PD + 256 : 8 * PD + 260]
            idh = c16[:, 8 * PD + 260 : 8 * PD + 388]
            oblk = c16[:, 8 * PD + 388 : 8 * PD + 420]
            b1v = c32[:, 0:4]
            thrv = c32[:, 4:5]
            selwv = c32[:, 5:37].rearrange("p (c a) -> p c a", c=8)
            dbv = c32[0:32, 37:38]
            dbv64 = c32[64:96, 37:38]
            selbv = c32[0:4, 38:39]
            o14v = c32[0:1, 39:43]
            i4v = c32[0:4, 43:47]
            e36v = c32[0:4, 47:83]

            # dummy ACT op so the gelu table set loads during startup DMAs
            dummy = cp.tile([1, 1], dt.float16, tag="dummy")
            nc.scalar.copy(dummy[:], idh[0:1, 0:1])

            # ---- adapter selector (once per core) ----
            xlast = xlast0
            ps_sel = pp.tile([4, 1], dt.float32, tag="small", bufs=2)
            for c in range(8):
                nc.tensor.matmul(
                    ps_sel[:], selwv[:, c, :], xlast[:, c : c + 1],
                    start=(c == 0), stop=(c == 7),
                )
            # t = tanh((z + sel_b)/2)  -> exp(z+sel_b) = (1+t)/(1-t)
            t4 = cp.tile([4, 1], dt.float32, tag="t4")
            nc.scalar.activation(t4[:], ps_sel[:], AF.Tanh, bias=selbv, scale=0.5)
            num4 = cp.tile([4, 1], dt.float32, tag="num4")
            nc.vector.tensor_scalar(num4[:], t4[:], 1.0, None, ALU.add)
            den4 = cp.tile([4, 1], dt.float32, tag="den4")
            nc.vector.tensor_scalar(den4[:], t4[:], -1.0, 1.0, ALU.mult, ALU.add)
            rden4 = cp.tile([4, 1], dt.float32, tag="rden4")
            nc.vector.reciprocal(rden4[:], den4[:])
            e4 = cp.tile([4, 1], dt.float32, tag="e4")
            nc.vector.tensor_mul(e4[:], num4[:], rden4[:])
            ps_et = pp.tile([1, 4], dt.float32, tag="small", bufs=2)
            nc.tensor.matmul(ps_et[:], e4[:], i4v, start=True, stop=True)
            ssum = cp.tile([1, 1], dt.float32, tag="ssum")
            nc.vector.reduce_sum(ssum[:], ps_et[:], axis=mybir.AxisListType.X)
            rsum = cp.tile([1, 1], dt.float32, tag="rsum")
            nc.vector.reciprocal(rsum[:], ssum[:])
            ps_rs = pp.tile([4, 1], dt.float32, tag="small", bufs=2)
            nc.tensor.matmul(ps_rs[:], o14v, rsum[:], start=True, stop=True)
            w4 = cp.tile([4, 1], dt.float32, tag="w4")
            nc.vector.tensor_tensor(w4[:], e4[:], ps_rs[:], ALU.mult)
            ps_wv = pp.tile([128, 1], dt.float32, tag="small", bufs=2)
            nc.tensor.matmul(ps_wv[0:KUP, :], e36v, w4[:], start=True, stop=True)
            nc.tensor.matmul(ps_wv[64 : 64 + KUP, :], e36v, w4[:], start=True, stop=True)
            wv_sb = cp.tile([128, 1], dt.float32, tag="wv")
            nc.scalar.copy(wv_sb[0:KUP, :], ps_wv[0:KUP, :])
            nc.scalar.copy(wv_sb[64 : 64 + KUP, :], ps_wv[64 : 64 + KUP, :])
            # fold adapter weights into the up matrix (rows 0:36 and 64:100)
            uw_sb = cp.tile([128, H], dt.float16, tag="uw")
            nc.vector.tensor_scalar(uw_sb[0:KUP, :], u_sb[0:KUP, :], wv_sb[0:KUP, :], None, ALU.mult)
            nc.vector.tensor_scalar(
                uw_sb[64 : 64 + KUP, :], u_sb[64 : 64 + KUP, :],
                wv_sb[64 : 64 + KUP, :], None, ALU.mult,
            )

            # ---- main loop over token tiles ----
            for i in range(N_TILES):
                if i == 0:
                    Xp = X0
                else:
                    Xp = wp.tile([128, 4, H], dt.float32, tag="X", name=f"X{i}", bufs=4)
                    nc.sync.dma_start(
                        out=Xp[:],
                        in_=x[i * T : (i + 1) * T, :].rearrange(
                            "(j p) h -> p j h", p=128
                        ),
                    )

                Xh = [
                    wp.tile([128, 2, H], dt.float16, tag="Xh", name=f"Xh{i}_{h}", bufs=4)
                    for h in range(2)
                ]
                for h in range(2):
                    for jj in range(2):
                        nc.vector.tensor_copy(
                            Xh[h][:, jj, :], Xp[:, 2 * h + jj, :]
                        )

                # transpose x -> XT[q][:, dc, :] (chunk c = 2q+dc), packed psum
                XT = []
                for q in range(4):
                    ps_xt = pp.tile([128, 2, T], dt.float16, tag="xt", name=f"psxt{i}_{q}")
                    for dc in range(2):
                        c = 2 * q + dc
                        for j in range(4):
                            nc.tensor.transpose(
                                ps_xt[:, dc, ts(j, 128)],
                                Xh[j // 2][:, j % 2, ts(c, 128)],
                                idh,
                            )
                    xt = wp.tile([128, 2, T], dt.float16, tag="XT", name=f"XT{i}_{q}", bufs=8)
                    nc.scalar.copy(xt[:, 0, :], ps_xt[:, 0, :])
                    nc.scalar.copy(xt[:, 1, :], ps_xt[:, 1, :])
                    XT.append(xt)

                def xtc(c):
                    return XT[c // 2][:, c % 2, :]

                # down-proj, col-packed: chunk c -> column group g=c%4 of the
                # PE array (concurrent in HW), two accumulation rounds, then a
                # ones-matmul reduces the 4 partition groups.
                ps_t4 = pp.tile([128, T], dt.float32, tag="small", name=f"pst4{i}", bufs=2)
                for r in range(2):
                    for g in range(4):
                        c = 4 * r + g
                        nc.tensor.matmul(
                            ps_t4[32 * g : 32 * g + 32, :], dcv[:, c, :], xtc(c),
                            start=(r == 0), stop=(r == 1),
                            tile_position=(0, 32 * g),
                        )
                t4sb = wp.tile([128, T], dt.float16, tag="t4sb", name=f"t4sb{i}", bufs=2)
                nc.scalar.copy(t4sb[:], ps_t4[:])

                # detector mm1 + gelu (emitted before the down fixup matmul so
                # the PE is not stalled waiting on the t4sb ACT copy)
                Hs = []
                for m in range(4):
                    ps_h = pp.tile([128, T], dt.float32, tag="h", name=f"psh{i}_{m}")
                    for c in range(8):
                        nc.tensor.matmul(
                            ps_h[:], w1v[:, c, ts(m, 128)], xtc(c),
                            start=(c == 0), stop=(c == 7),
                        )
                    hm = wp.tile([128, T], dt.float16, tag="Hs", name=f"Hs{i}_{m}", bufs=5)
                    nc.scalar.activation(
                        hm[:], ps_h[:], AF.Gelu, bias=b1v[:, m : m + 1]
                    )
                    Hs.append(hm)
                    if m == 0:
                        # down fixup: reduce the 4 column groups, into partition
                        # bases 0 and 64 (G duplicated for up row-packing)
                        ps_t = pp.tile([128, T], dt.float32, tag="small", name=f"pst{i}", bufs=2)
                        nc.tensor.matmul(ps_t[0:32, :], oblk, t4sb[:], start=True, stop=True)
                        nc.tensor.matmul(ps_t[64:96, :], oblk, t4sb[:], start=True, stop=True)
                        G = wp.tile([128, T], dt.float16, tag="G", name=f"G{i}", bufs=2)
                        nc.gpsimd.memset(G[32:KUP, :], 1.0)
                        nc.gpsimd.memset(G[64 + 32 : 64 + KUP, :], 1.0)
                        nc.scalar.activation(G[0:32, :], ps_t[0:32, :], AF.Gelu, bias=dbv)
                        nc.scalar.activation(
                            G[64:96, :], ps_t[64:96, :], AF.Gelu, bias=dbv64
                        )

                # detector mm2, natural orientation: z[tok,1] per token chunk j
                ps_z = pp.tile([128, 4], dt.float32, tag="small", name=f"psz{i}", bufs=2)
                for j in range(4):
                    for m in range(4):
                        nc.tensor.matmul(
                            ps_z[:, j : j + 1], Hs[m][:, ts(j, 128)],
                            w2v[:, m : m + 1],
                            start=(m == 0), stop=(m == 3),
                        )
                maskn = wp.tile([128, 4], dt.float32, tag="maskn", name=f"maskn{i}", bufs=2)
                nc.vector.tensor_scalar(maskn[:], ps_z[:], thrv, None, ALU.is_gt)

                # up-proj (natural layout) + fused mask*psum + residual + store
                # emit so adjacent matmuls alternate PE row groups (0 / 64)
                for jp in (0, 2):
                    for n in range(2):
                        for dj in range(2):
                            j = jp + dj
                            base = 64 * dj
                            ps_w = pp.tile(
                                [128, PD], dt.float32, tag="w",
                                name=f"psw{i}_{j}_{n}", bufs=2,
                            )
                            nc.tensor.matmul(
                                ps_w[:], G[base : base + KUP, ts(j, 128)],
                                uw_sb[base : base + KUP, ts(n, PD)],
                                start=True, stop=True,
                            )
                            nc.vector.scalar_tensor_tensor(
                                Xp[:, j, ts(n, PD)], ps_w[:], maskn[:, j : j + 1],
                                Xp[:, j, ts(n, PD)], ALU.mult, ALU.add,
                            )
                    if i == N_TILES - 1:
                        # last tile: store per (chunk, H-half) via HWDGE (short tail)
                        for jj in (jp, jp + 1):
                            for nn in range(2):
                                nc.sync.dma_start(
                                    out=out[
                                        i * T + jj * 128 : i * T + (jj + 1) * 128,
                                        nn * PD : (nn + 1) * PD,
                                    ],
                                    in_=Xp[:, jj, ts(nn, PD)],
                                )
                    else:
                        h = jp // 2
                        nc.gpsimd.dma_start(
                            out=out[
                                i * T + h * 256 : i * T + (h + 1) * 256, :
                            ].rearrange("(j p) h -> p j h", p=128),
                            in_=Xp[:, 2 * h : 2 * h + 2, :],
                        )

    nc.compile()
    return nc


_CACHE = {}


def _get_nc():
    if "nc" not in _CACHE:
        _CACHE["nc"] = _build()
    return _CACHE["nc"]


def _host_params(inputs):
    f32 = np.float32
    f16 = np.float16
    pd_w1 = np.asarray(inputs["pd_w1"], f32)          # [H, PD]
    pd_b1 = np.asarray(inputs["pd_b1"], f32)          # [PD]
    pd_w2 = np.asarray(inputs["pd_w2"], f32)          # [PD, 1]
    pd_b2 = np.asarray(inputs["pd_b2"], f32)          # [1]
    down_w = np.asarray(inputs["down_w"], f32)        # [A, H, d]
    down_b = np.asarray(inputs["down_b"], f32)        # [A, d]
    up_w = np.asarray(inputs["up_w"], f32)            # [A, d, H]
    up_b = np.asarray(inputs["up_b"], f32)            # [A, H]
    sel_w = np.asarray(inputs["sel_w"], f32)          # [H, A]
    sel_b = np.asarray(inputs["sel_b"], f32)          # [A]

    # f16 blob: w1 | dcat | w2 | idh
    w1s = pd_w1.reshape(8, 128, PD).transpose(1, 0, 2).reshape(128, 8 * PD)
    dcat = down_w.transpose(1, 0, 2).reshape(H, 32)
    dcats = dcat.reshape(8, 128, 32).transpose(1, 0, 2).reshape(128, 256)
    w2s = pd_w2.reshape(4, 128).T
    onesblk = np.tile(np.eye(32), (4, 1))  # [128, 32]
    fb16 = np.concatenate([w1s, dcats, w2s, np.eye(128), onesblk], axis=1).astype(f16)
    assert fb16.shape == (128, _F16_COLS)

    # f32 blob: b1(4) | thr(1) | selw(32) | db(1) | selb(1) | o14(4) | i4(4) | e36(36)
    b1s = pd_b1.reshape(4, 128).T
    thr = np.full((128, 1), math.log(THRESH / (1.0 - THRESH)) - float(pd_b2[0]), f32)
    selws = sel_w.reshape(8, 128, 4).transpose(1, 0, 2).reshape(128, 32)
    dbcol = np.zeros((128, 1), f32)
    dbcol[0:32, 0] = down_b.reshape(32)
    dbcol[64:96, 0] = down_b.reshape(32)
    selbcol = np.zeros((128, 1), f32)
    selbcol[0:4, 0] = sel_b / 2.0
    o14 = np.zeros((128, 4), f32)
    o14[0, :] = 1.0
    i4m = np.zeros((128, 4), f32)
    i4m[0:4, :] = np.eye(4)
    e36m = np.zeros((128, KUP), f32)
    for r in range(32):
        e36m[r // 8, r] = 1.0
    for a in range(4):
        e36m[a, 32 + a] = 1.0
    fb32 = np.concatenate(
        [b1s, thr, selws, dbcol, selbcol, o14, i4m, e36m], axis=1
    ).astype(f32)
    assert fb32.shape == (128, _F32_COLS)

    u36 = np.zeros((128, H), f16)
    u36[0:KUP] = np.concatenate(
        [SCALE * up_w.reshape(32, H), SCALE * up_b], axis=0
    ).astype(f16)
    u36[64 : 64 + KUP] = u36[0:KUP]
    return dict(fb16=fb16, fb32=fb32, u36=u36)


def _run(inputs, trace=False, **kwargs):
    nc = _get_nc()
    params = _host_params(inputs)
    hs = np.asarray(inputs["hidden_states"], np.float32)
    in_maps = [dict(params, x=np.ascontiguousarray(hs[b])) for b in range(N_CORES)]
    try:
        res = run_bass_kernel_spmd(
            nc, in_maps, core_ids=list(range(N_CORES)), trace=trace, **kwargs
        )
    except ModuleNotFoundError:
        res = run_bass_kernel_spmd(
            nc, in_maps, core_ids=list(range(N_CORES)), trace=False, **kwargs
        )
    out = np.stack([res.results[b]["out"] for b in range(N_CORES)], axis=0)
    return out.astype(np.float32), res


def kernel(**inputs) -> np.ndarray:
    out, _ = _run(inputs, trace=False)
    return out



# revision 8
# speedup vs baseline: 1.6168x; 1.6168x over previous
"""Trainium2 Bass kernel for nn_CRFTModule (moe_routing).

Pure data parallel over batch: 8 cores, one batch row (4096 tokens) each.

Math per core (batch b, S=4096 tokens, H=1024):
  z      = gelu(x @ W1 + b1) @ W2                    (critical-path detector)
  mask   = z > logit(0.7) - b2                       (compare in logit space)
  aw     = softmax(x[last] @ sel_w + sel_b)          (adapter selector, 4-way)
  t      = gelu(x @ Dcat + db)                       (all 4 down-projs, [S,32])
  out    = x + mask * (sum_a 0.3*aw[a] (t_a @ up_w[a] + up_b[a]))

Layout strategy (everything transposed, fp8-heavy):
  - Host pre-transposes x to x^T [H, S] and ships it as an fp8 e4m3 hi/lo
    pair (hi = fp8(x), lo = fp8(x - hi); hi+lo reconstructs x to ~7e-4 rel,
    same byte count as f16).  Tiles of 512 tokens: [128, 8 chunks, 2, 512].
  - Detector mm1 and the down-proj run as fp8 DoubleRow matmuls (K=256 per
    pass) on the hi planes; W1/Dcat are pre-scaled by 8 on the host with
    1/8 folded into the gelu activation's scale argument.
  - mm2 (z = h @ W2) stays f16 with W2 replicated across 32 columns, so the
    psum is the z row broadcast to 32 partitions; the mask is taken with a
    DVE is_gt and folded into G (gelu(down) + ones rows) BEFORE the up-proj.
  - The residual add runs on the PE: each up-proj psum group starts with a
    16*I DoubleRow identity matmul that sums 16*(hi+lo) into psum; the
    up weights are pre-scaled by 0.3*16, so the drain is a pure psum*(1/16)
    copy to f16, split across ACT/DVE/GPSIMD.
  - Output is written transposed f16 [8, 128, 8, 512]; the host transposes
    back and upcasts.
"""
import math

import numpy as np
import ml_dtypes

import concourse.bacc as bacc
import concourse.mybir as mybir
from concourse.tile import TileContext
from concourse.bass_utils import run_bass_kernel_spmd

dt = mybir.dt
AF = mybir.ActivationFunctionType
ALU = mybir.AluOpType
DR = mybir.MatmulPerfMode.DoubleRow

B, S, H = 8, 4096, 1024
A_DIM, N_ADAPT = 8, 4
PD = H // 2
T = 512
NT = S // T
N_CORES = 8
THRESH, SCALE = 0.7, 0.3
WS = 8.0      # host prescale on W1/Dcat (fp8 range); 1/WS folded into gelu scale
US = 16.0     # host prescale on up weights + residual identity; drain scales 1/US

F8 = ml_dtypes.float8_e4m3

# f32 const blob columns: b1(0:4) db(4:5) thr(5:6) selb2(6:7) o14(7:11)
# i4(11:15) e32(15:47) e32b(47:79)
_CF_COLS = 79
# f16 const blob columns: selw(0:32) xlast(32:40)
_CS_COLS = 40

# drain engine per H-chunk (GPSIMD cannot read PSUM): ACT has the gelus too
_DRAIN_ENG = ["act", "act", "act", "dve", "dve", "dve", "dve", "dve"]


def _build():
    nc = bacc.Bacc("TRN2", target_bir_lowering=False, debug=False)

    x8 = nc.declare_dram_parameter("x8", [NT, 128, 8, 2, T], dt.float8e4, isOutput=False)
    w1 = nc.declare_dram_parameter("w1", [128, 4, 4, 2, 128], dt.float8e4, isOutput=False)
    dw = nc.declare_dram_parameter("dw", [128, 4, 2, 32], dt.float8e4, isOutput=False)
    ii = nc.declare_dram_parameter("ii", [128, 2, 128], dt.float8e4, isOutput=False)
    w2r = nc.declare_dram_parameter("w2r", [128, 4, 32], dt.float16, isOutput=False)
    u36 = nc.declare_dram_parameter("u36", [32, 2, H], dt.float16, isOutput=False)
    cf = nc.declare_dram_parameter("cf", [128, _CF_COLS], dt.float32, isOutput=False)
    cs = nc.declare_dram_parameter("cs", [128, _CS_COLS], dt.float16, isOutput=False)
    out = nc.declare_dram_parameter("out", [NT, 128, 8, T], dt.float16, isOutput=True)

    with TileContext(nc) as tc:
        with (
            tc.tile_pool(name="consts", bufs=1) as cp,
            tc.tile_pool(name="work", bufs=2) as wp,
            tc.tile_pool(name="psum", bufs=2, space="PSUM") as pp,
        ):
            # ---- constant loads (small first, big w1 before the tile loop) ----
            cs_sb = cp.tile([128, _CS_COLS], dt.float16, tag="cs")
            nc.sync.dma_start(out=cs_sb[:], in_=cs[:])
            cf_sb = cp.tile([128, _CF_COLS], dt.float32, tag="cf")
            nc.sync.dma_start(out=cf_sb[:], in_=cf[:])
            w2_sb = cp.tile([128, 4, 32], dt.float16, tag="w2")
            nc.sync.dma_start(out=w2_sb[:], in_=w2r[:])
            dw_sb = cp.tile([128, 4, 2, 32], dt.float8e4, tag="dw")
            nc.sync.dma_start(out=dw_sb[:], in_=dw[:])
            ii_sb = cp.tile([128, 2, 128], dt.float8e4, tag="ii")
            nc.sync.dma_start(out=ii_sb[:], in_=ii[:])
            u36_sb = cp.tile([32, 2, H], dt.float16, tag="u36")
            nc.sync.dma_start(out=u36_sb[:], in_=u36[:])
            w1_sb = cp.tile([128, 4, 4, 2, 128], dt.float8e4, tag="w1")
            nc.sync.dma_start(out=w1_sb[:], in_=w1[:])

            b1v = cf_sb[:, 0:4]
            dbv = cf_sb[0:32, 4:5]
            thrv = cf_sb[0:32, 5:6]
            selb2 = cf_sb[0:4, 6:7]
            o14v = cf_sb[0:1, 7:11]
            i4v = cf_sb[0:4, 11:15]
            e32v = cf_sb[0:4, 15:47]
            e32bv = cf_sb[0:4, 47:79]

            # dummy ACT op so the gelu/tanh table set is resident early
            dummy = cp.tile([1, 1], dt.float16, tag="dummy")
            nc.scalar.copy(dummy[:], cs_sb[0:1, 0:1])

            # ---- adapter selector (once per core) -> fold into up weights ----
            # selector psums borrow the "hps" tag (bank-granular allocator;
            # they rotate through the same 2 banks before the main loop)
            ps_sel = pp.tile([128, T], dt.float32, tag="hps", name="ps_sel", bufs=2)[0:4, 0:1]
            for c in range(8):
                nc.tensor.matmul(
                    ps_sel, cs_sb[:, 4 * c : 4 * c + 4], cs_sb[:, 32 + c : 33 + c],
                    start=(c == 0), stop=(c == 7),
                )
            # softmax via tanh identity: exp(z+b) = (1+t)/(1-t), t=tanh((z+b)/2)
            t4 = cp.tile([4, 1], dt.float32, tag="t4")
            nc.scalar.activation(t4[:], ps_sel, AF.Tanh, bias=selb2, scale=0.5)
            num4 = cp.tile([4, 1], dt.float32, tag="num4")
            nc.vector.tensor_scalar(num4[:], t4[:], 1.0, None, ALU.add)
            den4 = cp.tile([4, 1], dt.float32, tag="den4")
            nc.vector.tensor_scalar(den4[:], t4[:], -1.0, 1.0, ALU.mult, ALU.add)
            rden4 = cp.tile([4, 1], dt.float32, tag="rden4")
            nc.vector.reciprocal(rden4[:], den4[:])
            e4 = cp.tile([4, 1], dt.float32, tag="e4")
            nc.vector.tensor_tensor(e4[:], num4[:], rden4[:], ALU.mult)
            ps_et = pp.tile([128, T], dt.float32, tag="hps", name="ps_et", bufs=2)[0:1, 0:4]
            nc.tensor.matmul(ps_et, e4[:], i4v, start=True, stop=True)
            ssum = cp.tile([1, 1], dt.float32, tag="ssum")
            nc.vector.reduce_sum(ssum[:], ps_et, axis=mybir.AxisListType.X)
            rsum = cp.tile([1, 1], dt.float32, tag="rsum")
            nc.vector.reciprocal(rsum[:], ssum[:])
            ps_rs = pp.tile([128, T], dt.float32, tag="hps", name="ps_rs", bufs=2)[0:4, 0:1]
            nc.tensor.matmul(ps_rs, o14v, rsum[:], start=True, stop=True)
            w4 = cp.tile([4, 1], dt.float32, tag="w4")
            nc.vector.tensor_tensor(w4[:], e4[:], ps_rs, ALU.mult)
            # wv32 = per-row adapter weight for up_w rows; wvB for up_b rows
            ps_w32 = pp.tile([128, T], dt.float32, tag="hps", name="ps_w32", bufs=2)[0:32, 0:1]
            nc.tensor.matmul(ps_w32, e32v, w4[:], start=True, stop=True)
            ps_wB = pp.tile([128, T], dt.float32, tag="hps", name="ps_wB", bufs=2)[0:32, 0:1]
            nc.tensor.matmul(ps_wB, e32bv, w4[:], start=True, stop=True)
            wv32 = cp.tile([32, 1], dt.float32, tag="wv32")
            nc.vector.tensor_copy(wv32[:], ps_w32)
            wvB = cp.tile([32, 1], dt.float32, tag="wvB")
            nc.vector.tensor_copy(wvB[:], ps_wB)
            uw8 = cp.tile([32, 2, H], dt.float8e4, tag="uw8")
            nc.vector.tensor_scalar(uw8[:, 0, :], u36_sb[:, 0, :], wv32[:], None, ALU.mult)
            nc.vector.tensor_scalar(uw8[:, 1, :], u36_sb[:, 1, :], wvB[:], None, ALU.mult)

            # masked-G double buffers; the zero rows are written once
            gms = [
                cp.tile([32, 2, T], dt.float8e4, tag=f"gm{k}", name=f"gm{k}")
                for k in range(2)
            ]
            for k in range(2):
                nc.gpsimd.memset(gms[k][0:32, 1, :], 0.0)

            # ---- software-pipelined main loop ----
            prev = None  # (xt, gm) of tile i-1
            for i in range(NT + 1):
                if i < NT:
                    xt = wp.tile([128, 8, 2, T], dt.float8e4, tag="x8", name=f"x{i}", bufs=3)
                    nc.sync.dma_start(out=xt[:], in_=x8[i])

                    # detector mm1 (fp8 DoubleRow on hi planes) + gelu
                    hsb = wp.tile([128, 4, T], dt.float16, tag="h", name=f"h{i}", bufs=2)
                    for m in range(4):
                        psh = pp.tile([128, T], dt.float32, tag="hps", name=f"hps{i}_{m}", bufs=2)
                        for p in range(4):
                            nc.tensor.matmul(
                                psh[:], w1_sb[:, m, p, :, :], xt[:, 2 * p : 2 * p + 2, 0, :],
                                start=(p == 0), stop=(p == 3), perf_mode=DR,
                            )
                        nc.scalar.activation(
                            hsb[:, m, :], psh[:], AF.Gelu,
                            bias=b1v[:, m : m + 1], scale=1.0 / WS,
                        )

                    # down-proj (fp8 DoubleRow) + gelu
                    pst = pp.tile([32, T], dt.float32, tag="tps", name=f"tps{i}", bufs=1)
                    for p in range(4):
                        nc.tensor.matmul(
                            pst[:], dw_sb[:, p, :, :], xt[:, 2 * p : 2 * p + 2, 0, :],
                            start=(p == 0), stop=(p == 3), perf_mode=DR,
                        )
                    g0 = wp.tile([32, T], dt.float16, tag="g0", name=f"g0{i}", bufs=2)
                    nc.scalar.activation(g0[:], pst[:], AF.Gelu, bias=dbv, scale=1.0 / WS)

                    # detector mm2 (f16, W2 replicated to 32 cols) -> mask
                    psz = pp.tile([32, T], dt.float32, tag="zps", name=f"zps{i}", bufs=1)
                    for p in range(4):
                        nc.tensor.matmul(
                            psz[:], w2_sb[:, p, :], hsb[:, p, :],
                            start=(p == 0), stop=(p == 3),
                        )
                    msk = wp.tile([32, T], dt.float16, tag="msk", name=f"msk{i}", bufs=2)
                    nc.vector.tensor_scalar(msk[:], psz[:], thrv, None, ALU.is_gt)
                    gm = gms[i % 2]
                    nc.vector.tensor_tensor(gm[:, 0, :], g0[:], msk[:], ALU.mult)
                    nc.gpsimd.tensor_copy(gm[0:4, 1, :], msk[0:4, :])
                    cur = (xt, gm)
                else:
                    cur = None

                if prev is not None:
                    xtp, gmp = prev
                    j = i - 1
                    osb = wp.tile([128, 8, T], dt.float16, tag="o", name=f"o{j}", bufs=2)
                    for c in range(8):
                        psu = pp.tile([128, T], dt.float32, tag="ups", name=f"ups{j}_{c}", bufs=2)
                        # residual: 16*(hi+lo) via DoubleRow identity
                        nc.tensor.matmul(
                            psu[:], ii_sb[:], xtp[:, c, :, :],
                            start=True, stop=False, perf_mode=DR,
                        )
                        # up-proj, mask+softmax+0.3*16 already folded in
                        nc.tensor.matmul(
                            psu[:], uw8[:, :, 128 * c : 128 * (c + 1)], gmp[:],
                            start=False, stop=True, perf_mode=DR,
                        )
                        eng = _DRAIN_ENG[c]
                        if eng == "act":
                            nc.scalar.mul(osb[:, c, :], psu[:], 1.0 / US)
                        elif eng == "dve":
                            nc.vector.tensor_scalar(osb[:, c, :], psu[:], 1.0 / US, None, ALU.mult)
                        else:
                            nc.gpsimd.tensor_scalar(osb[:, c, :], psu[:], 1.0 / US, None, ALU.mult)
                    nc.sync.dma_start(out=out[j], in_=osb[:])
                prev = cur

    nc.compile()
    return nc


_CACHE = {}


def _get_nc():
    if "nc" not in _CACHE:
        _CACHE["nc"] = _build()
    return _CACHE["nc"]


def _host_shared(inputs):
    f32, f16 = np.float32, np.float16
    pd_w1 = np.asarray(inputs["pd_w1"], f32)
    pd_b1 = np.asarray(inputs["pd_b1"], f32)
    pd_w2 = np.asarray(inputs["pd_w2"], f32)
    pd_b2 = np.asarray(inputs["pd_b2"], f32)
    down_w = np.asarray(inputs["down_w"], f32)
    down_b = np.asarray(inputs["down_b"], f32)
    up_w = np.asarray(inputs["up_w"], f32)
    up_b = np.asarray(inputs["up_b"], f32)
    sel_w = np.asarray(inputs["sel_w"], f32)
    sel_b = np.asarray(inputs["sel_b"], f32)

    # w1_dr[k, m, p, i, j] = WS * W1[128*(2p+i)+k, 128m+j]
    w1_dr = np.ascontiguousarray(
        (pd_w1 * WS).reshape(4, 2, 128, 4, 128).transpose(2, 3, 0, 1, 4)
    ).astype(F8)
    # dcat[h, a*8+d] = down_w[a, h, d]
    dcat = down_w.transpose(1, 0, 2).reshape(H, 32) * WS
    dw_dr = np.ascontiguousarray(
        dcat.reshape(4, 2, 128, 32).transpose(2, 0, 1, 3)
    ).astype(F8)
    # w2 replicated across 32 cols: w2r[k, p, j] = w2[128p+k]
    w2rep = np.ascontiguousarray(
        np.repeat(pd_w2.reshape(4, 128).T[:, :, None], 32, axis=2)
    ).astype(f16)
    iim = np.zeros((128, 2, 128), f32)
    iim[np.arange(128), 0, np.arange(128)] = US
    iim[np.arange(128), 1, np.arange(128)] = US
    iim = iim.astype(F8)
    u36m = np.zeros((32, 2, H), f32)
    u36m[:, 0, :] = SCALE * US * up_w.reshape(32, H)
    u36m[0:4, 1, :] = SCALE * US * up_b
    u36m = u36m.astype(f16)

    cfm = np.zeros((128, _CF_COLS), f32)
    cfm[:, 0:4] = pd_b1.reshape(4, 128).T
    cfm[0:32, 4] = down_b.reshape(32)
    cfm[:, 5] = math.log(THRESH / (1.0 - THRESH)) - float(pd_b2[0])
    cfm[0:4, 6] = sel_b / 2.0
    cfm[0, 7:11] = 1.0
    cfm[0:4, 11:15] = np.eye(4)
    for r in range(32):
        cfm[r // 8, 15 + r] = 1.0
    cfm[0:4, 47:51] = np.eye(4)

    selw = np.ascontiguousarray(
        sel_w.reshape(8, 128, 4).transpose(1, 0, 2).reshape(128, 32)
    ).astype(f16)
    return dict(w1=w1_dr, dw=dw_dr, w2r=w2rep, ii=iim, u36=u36m, cf=cfm), selw


def _host_core(xb, selw):
    """Per-core inputs from one batch row xb [S, H] f32."""
    xt = np.ascontiguousarray(xb.T)                     # [H, S]
    hi = xt.astype(F8)
    lo = (xt - hi.astype(np.float32)).astype(F8)
    x8m = np.stack([hi, lo], 0)                         # [s, H, S]
    x8m = x8m.reshape(2, 8, 128, NT, T)                 # [s, c, p, i, t]
    x8m = np.ascontiguousarray(x8m.transpose(3, 2, 1, 0, 4))  # [i, p, c, s, t]
    csm = np.zeros((128, _CS_COLS), np.float16)
    csm[:, 0:32] = selw
    csm[:, 32:40] = xb[-1].reshape(8, 128).T.astype(np.float16)
    return dict(x8=x8m, cs=csm)


def _run(inputs, trace=False, **kwargs):
    nc = _get_nc()
    shared, selw = _host_shared(inputs)
    hs = np.asarray(inputs["hidden_states"], np.float32)
    in_maps = [dict(shared, **_host_core(hs[b], selw)) for b in range(N_CORES)]
    try:
        res = run_bass_kernel_spmd(
            nc, in_maps, core_ids=list(range(N_CORES)), trace=trace, **kwargs
        )
    except ModuleNotFoundError:
        res = run_bass_kernel_spmd(
            nc, in_maps, core_ids=list(range(N_CORES)), trace=False, **kwargs
        )
    outs = []
    for b in range(N_CORES):
        ob = np.asarray(res.results[b]["out"])          # [NT, 128, 8, T] f16
        outs.append(ob.transpose(0, 3, 2, 1).reshape(S, H).astype(np.float32))
    return np.stack(outs, 0), res


def kernel(**inputs) -> np.ndarray:
    out, _ = _run(inputs, trace=False)
    return out


# revision 17
# speedup vs baseline: 2.0502x; 1.2681x over previous
"""Trainium2 Bass kernel for nn_CRFTModule (moe_routing).

Pure data parallel over batch: 8 cores, one batch row (4096 tokens) each.

Math per core (batch b, S=4096 tokens, H=1024):
  z      = gelu(x @ W1 + b1) @ W2                    (critical-path detector)
  mask   = z > logit(0.7) - b2                       (compare in logit space)
  aw     = softmax(x[last] @ sel_w + sel_b)          (adapter selector, 4-way)
  t      = gelu(x @ Dcat + db)                       (all 4 down-projs, [S,32])
  out    = x + mask * (sum_a 0.3*aw[a] (t_a @ up_w[a] + up_b[a]))

Layout strategy (everything transposed, fp8-heavy):
  - Host pre-transposes x to x^T [H, S] and ships it as an fp8 e4m3 hi/lo
    pair (hi = fp8(x), lo = fp8(x - hi); hi+lo reconstructs x to ~7e-4 rel,
    same byte count as f16).  Tiles of 512 tokens: [128, 8 chunks, 2, 512].
  - Detector mm1 and the down-proj run as fp8 DoubleRow matmuls (K=256 per
    pass) on the hi planes; W1/Dcat are pre-scaled by 8 on the host with
    1/8 folded into the gelu activation's scale argument.
  - mm2 (z = h @ W2) stays f16 with W2 replicated across 32 columns, so the
    psum is the z row broadcast to 32 partitions; the mask is taken with a
    DVE is_gt and folded into G (gelu(down) + ones rows) BEFORE the up-proj.
  - The residual add runs on the PE: each up-proj psum group starts with a
    16*I DoubleRow identity matmul that sums 16*(hi+lo) into psum; the
    up weights are pre-scaled by 0.3*16, so the drain is a pure psum*(1/16)
    copy to f16, split across ACT/DVE/GPSIMD.
  - Output is written transposed f16 [8, 128, 8, 512]; the host transposes
    back and upcasts.
"""
import math

import numpy as np
import ml_dtypes

import concourse.bacc as bacc
import concourse.mybir as mybir
from concourse.tile import TileContext
from concourse.bass_utils import run_bass_kernel_spmd

dt = mybir.dt
AF = mybir.ActivationFunctionType
ALU = mybir.AluOpType
DR = mybir.MatmulPerfMode.DoubleRow

B, S, H = 8, 4096, 1024
A_DIM, N_ADAPT = 8, 4
PD = H // 2
T = 512
NT = S // T
N_CORES = 8
THRESH, SCALE = 0.7, 0.3
WS = 8.0      # host prescale on W1/Dcat (fp8 range); 1/WS folded into gelu scale
US = 16.0     # host prescale on up weights + residual identity; drain scales 1/US

F8 = ml_dtypes.float8_e4m3

# f32 const blob columns: b1(0:4) db(4:5) thr(5:6) selb2(6:7) o14(7:11)
# i4(11:15) e32(15:47) e32b(47:79)
_CF_COLS = 79
# f16 const blob columns: selw(0:32) xlast(32:40)
_CS_COLS = 40

# drain engine per H-chunk (GPSIMD cannot read PSUM): ACT has the gelus too
_DRAIN_ENG = ["act", "act", "act", "dve", "dve", "dve", "dve", "dve"]


def _build():
    nc = bacc.Bacc("TRN2", target_bir_lowering=False, debug=False)

    x8 = nc.declare_dram_parameter("x8", [NT, 128, 8, 2, T], dt.float8e4, isOutput=False)
    w1 = nc.declare_dram_parameter("w1", [128, 4, 4, 2, 128], dt.float8e4, isOutput=False)
    dw = nc.declare_dram_parameter("dw", [128, 4, 2, 32], dt.float8e4, isOutput=False)
    ii = nc.declare_dram_parameter("ii", [128, 2, 128], dt.float8e4, isOutput=False)
    w2r = nc.declare_dram_parameter("w2r", [128, 2, 2, 32], dt.float8e4, isOutput=False)
    u36 = nc.declare_dram_parameter("u36", [32, 2, H], dt.float16, isOutput=False)
    cf = nc.declare_dram_parameter("cf", [128, _CF_COLS], dt.float32, isOutput=False)
    cs = nc.declare_dram_parameter("cs", [128, _CS_COLS], dt.float16, isOutput=False)
    out = nc.declare_dram_parameter("out", [NT, 128, 8, T], dt.float16, isOutput=True)

    with TileContext(nc) as tc:
        with (
            tc.tile_pool(name="consts", bufs=1) as cp,
            tc.tile_pool(name="work", bufs=2) as wp,
            tc.tile_pool(name="psum", bufs=2, space="PSUM") as pp,
        ):
            # ---- constant loads (small first, big w1 before the tile loop) ----
            cs_sb = cp.tile([128, _CS_COLS], dt.float16, tag="cs")
            nc.sync.dma_start(out=cs_sb[:], in_=cs[:])
            cf_sb = cp.tile([128, _CF_COLS], dt.float32, tag="cf")
            nc.sync.dma_start(out=cf_sb[:], in_=cf[:])
            w2_sb = cp.tile([128, 2, 2, 32], dt.float8e4, tag="w2")
            nc.sync.dma_start(out=w2_sb[:], in_=w2r[:])
            dw_sb = cp.tile([128, 4, 2, 32], dt.float8e4, tag="dw")
            nc.sync.dma_start(out=dw_sb[:], in_=dw[:])
            ii_sb = cp.tile([128, 2, 128], dt.float8e4, tag="ii")
            nc.sync.dma_start(out=ii_sb[:], in_=ii[:])
            u36_sb = cp.tile([32, 2, H], dt.float16, tag="u36")
            nc.sync.dma_start(out=u36_sb[:], in_=u36[:])
            xt0 = wp.tile([128, 8, 2, T], dt.float8e4, tag="x8", name="x0", bufs=6)
            nc.sync.dma_start(out=xt0[:], in_=x8[0])
            w1_sb = cp.tile([128, 4, 4, 2, 128], dt.float8e4, tag="w1")
            nc.sync.dma_start(out=w1_sb[:], in_=w1[:])

            b1v = cf_sb[:, 0:4]
            dbv = cf_sb[0:32, 4:5]
            thrv = cf_sb[0:32, 5:6]
            selb2 = cf_sb[0:4, 6:7]
            o14v = cf_sb[0:1, 7:11]
            i4v = cf_sb[0:4, 11:15]
            e32v = cf_sb[0:4, 15:47]
            e32bv = cf_sb[0:4, 47:79]

            # dummy ACT op so the gelu/tanh table set is resident early
            dummy = cp.tile([1, 1], dt.float16, tag="dummy")
            nc.scalar.copy(dummy[:], cs_sb[0:1, 0:1])

            # ---- adapter selector (once per core) -> fold into up weights ----
            # selector psums borrow the "hps" tag (bank-granular allocator;
            # they rotate through the same 2 banks before the main loop)
            ps_sel = pp.tile([128, T], dt.float32, tag="hps", name="ps_sel", bufs=2)[0:4, 0:1]
            for c in range(8):
                nc.tensor.matmul(
                    ps_sel, cs_sb[:, 4 * c : 4 * c + 4], cs_sb[:, 32 + c : 33 + c],
                    start=(c == 0), stop=(c == 7),
                )
            # softmax via tanh identity: exp(z+b) = (1+t)/(1-t), t=tanh((z+b)/2)
            t4 = cp.tile([4, 1], dt.float32, tag="t4")
            nc.scalar.activation(t4[:], ps_sel, AF.Tanh, bias=selb2, scale=0.5)
            num4 = cp.tile([4, 1], dt.float32, tag="num4")
            nc.vector.tensor_scalar(num4[:], t4[:], 1.0, None, ALU.add)
            den4 = cp.tile([4, 1], dt.float32, tag="den4")
            nc.vector.tensor_scalar(den4[:], t4[:], -1.0, 1.0, ALU.mult, ALU.add)
            rden4 = cp.tile([4, 1], dt.float32, tag="rden4")
            nc.vector.reciprocal(rden4[:], den4[:])
            e4 = cp.tile([4, 1], dt.float32, tag="e4")
            nc.vector.tensor_tensor(e4[:], num4[:], rden4[:], ALU.mult)
            ps_et = pp.tile([128, T], dt.float32, tag="hps", name="ps_et", bufs=2)[0:1, 0:4]
            nc.tensor.matmul(ps_et, e4[:], i4v, start=True, stop=True)
            ssum = cp.tile([1, 1], dt.float32, tag="ssum")
            nc.vector.reduce_sum(ssum[:], ps_et, axis=mybir.AxisListType.X)
            rsum = cp.tile([1, 1], dt.float32, tag="rsum")
            nc.vector.reciprocal(rsum[:], ssum[:])
            ps_rs = pp.tile([128, T], dt.float32, tag="hps", name="ps_rs", bufs=2)[0:4, 0:1]
            nc.tensor.matmul(ps_rs, o14v, rsum[:], start=True, stop=True)
            w4 = cp.tile([4, 1], dt.float32, tag="w4")
            nc.vector.tensor_tensor(w4[:], e4[:], ps_rs, ALU.mult)
            # wv32 = per-row adapter weight for up_w rows; wvB for up_b rows
            ps_w32 = pp.tile([128, T], dt.float32, tag="hps", name="ps_w32", bufs=2)[0:32, 0:1]
            nc.tensor.matmul(ps_w32, e32v, w4[:], start=True, stop=True)
            ps_wB = pp.tile([128, T], dt.float32, tag="hps", name="ps_wB", bufs=2)[0:32, 0:1]
            nc.tensor.matmul(ps_wB, e32bv, w4[:], start=True, stop=True)
            wv32 = cp.tile([32, 1], dt.float32, tag="wv32")
            nc.vector.tensor_copy(wv32[:], ps_w32)
            wvB = cp.tile([32, 1], dt.float32, tag="wvB")
            nc.vector.tensor_copy(wvB[:], ps_wB)
            uw8 = cp.tile([32, 2, H], dt.float8e4, tag="uw8")
            nc.vector.tensor_scalar(uw8[:, 0, :], u36_sb[:, 0, :], wv32[:], None, ALU.mult)
            nc.vector.tensor_scalar(uw8[:, 1, :], u36_sb[:, 1, :], wvB[:], None, ALU.mult)

            # masked-G double buffers; the zero rows are written once
            gms = [
                cp.tile([32, 2, T], dt.float8e4, tag=f"gm{k}", name=f"gm{k}")
                for k in range(3)
            ]
            for k in range(3):
                nc.gpsimd.memset(gms[k][0:32, 1, :], 0.0)

            # ---- software-pipelined main loop (3 stages deep) ----
            # stage B(i-2) (up-proj + residual + drain + store) is emitted
            # before stage A(i); the up stage lags two tiles so the detector
            # chain of tile i overlaps two full DMA slots
            hist = {}
            for i in range(NT + 2):
                if i - 2 >= 0:
                    xtp, gmp = hist.pop(i - 2)
                    j = i - 2
                    osb = wp.tile([128, 8, T], dt.float16, tag="o", name=f"o{j}", bufs=3)
                    for q in range(4):  # H-chunk pairs; one drain op per pair
                        psu = pp.tile([128, 2, T], dt.float32, tag="ups", name=f"ups{j}_{q}", bufs=2)
                        for d in range(2):
                            c = 2 * q + d
                            # residual: 16*(hi+lo) via DoubleRow identity
                            nc.tensor.matmul(
                                psu[:, d, :], ii_sb[:], xtp[:, c, :, :],
                                start=True, stop=False, perf_mode=DR,
                            )
                            # up-proj, mask+softmax+0.3*16 already folded in
                            nc.tensor.matmul(
                                psu[:, d, :], uw8[:, :, 128 * c : 128 * (c + 1)], gmp[:],
                                start=False, stop=True, perf_mode=DR,
                            )
                        if q == 0:
                            nc.scalar.mul(osb[:, 2 * q : 2 * q + 2, :], psu[:], 1.0 / US)
                        else:
                            nc.vector.tensor_scalar(
                                osb[:, 2 * q : 2 * q + 2, :], psu[:], 1.0 / US, None, ALU.mult
                            )
                    # SWDGE path: a waiting store must not block later loads
                    # queued behind it on the SP sequencer; the last store goes
                    # in halves so its first half overlaps the final drains
                    if j == NT - 1:
                        nc.gpsimd.dma_start(out=out[j, :, 0:4, :], in_=osb[:, 0:4, :])
                        nc.gpsimd.dma_start(out=out[j, :, 4:8, :], in_=osb[:, 4:8, :])
                    else:
                        nc.gpsimd.dma_start(out=out[j], in_=osb[:])

                if i < NT:
                    if i == 0:
                        xt = xt0
                    else:
                        xt = wp.tile([128, 8, 2, T], dt.float8e4, tag="x8", name=f"x{i}", bufs=6)
                        nc.sync.dma_start(out=xt[:], in_=x8[i])

                    # detector mm1 (fp8 DoubleRow on hi planes) + gelu -> fp8 h
                    hsb = wp.tile([128, 4, T], dt.float8e4, tag="h", name=f"h{i}", bufs=2)
                    for m in range(4):
                        psh = pp.tile([128, T], dt.float32, tag="hps", name=f"hps{i}_{m}", bufs=2)
                        for p in range(4):
                            nc.tensor.matmul(
                                psh[:], w1_sb[:, m, p, :, :], xt[:, 2 * p : 2 * p + 2, 0, :],
                                start=(p == 0), stop=(p == 3), perf_mode=DR,
                            )
                        nc.scalar.activation(
                            hsb[:, m, :], psh[:], AF.Gelu,
                            bias=b1v[:, m : m + 1], scale=1.0 / WS,
                        )

                    # down-proj (fp8 DoubleRow) + gelu
                    pst = pp.tile([32, T], dt.float32, tag="tps", name=f"tps{i}", bufs=1)
                    for p in range(4):
                        nc.tensor.matmul(
                            pst[:], dw_sb[:, p, :, :], xt[:, 2 * p : 2 * p + 2, 0, :],
                            start=(p == 0), stop=(p == 3), perf_mode=DR,
                        )
                    g0 = wp.tile([32, T], dt.float16, tag="g0", name=f"g0{i}", bufs=2)
                    nc.scalar.activation(g0[:], pst[:], AF.Gelu, bias=dbv, scale=1.0 / WS)

                    # detector mm2 (fp8 DoubleRow, W2*16 replicated to 32 cols);
                    # psum holds 16*z so the threshold is prescaled by 16 too
                    psz = pp.tile([32, T], dt.float32, tag="zps", name=f"zps{i}", bufs=1)
                    for p in range(2):
                        nc.tensor.matmul(
                            psz[:], w2_sb[:, p, :, :], hsb[:, 2 * p : 2 * p + 2, :],
                            start=(p == 0), stop=(p == 1), perf_mode=DR,
                        )
                    msk = wp.tile([32, T], dt.float16, tag="msk", name=f"msk{i}", bufs=2)
                    nc.vector.tensor_scalar(msk[:], psz[:], thrv, None, ALU.is_gt)
                    gm = gms[i % 3]
                    nc.gpsimd.tensor_tensor(gm[:, 0, :], g0[:], msk[:], ALU.mult)
                    nc.gpsimd.tensor_copy(gm[0:4, 1, :], msk[0:4, :])
                    hist[i] = (xt, gm)

    nc.compile()
    return nc


_CACHE = {}


def _get_nc():
    if "nc" not in _CACHE:
        _CACHE["nc"] = _build()
    return _CACHE["nc"]


def _host_shared(inputs):
    f32, f16 = np.float32, np.float16
    pd_w1 = np.asarray(inputs["pd_w1"], f32)
    pd_b1 = np.asarray(inputs["pd_b1"], f32)
    pd_w2 = np.asarray(inputs["pd_w2"], f32)
    pd_b2 = np.asarray(inputs["pd_b2"], f32)
    down_w = np.asarray(inputs["down_w"], f32)
    down_b = np.asarray(inputs["down_b"], f32)
    up_w = np.asarray(inputs["up_w"], f32)
    up_b = np.asarray(inputs["up_b"], f32)
    sel_w = np.asarray(inputs["sel_w"], f32)
    sel_b = np.asarray(inputs["sel_b"], f32)

    # w1_dr[k, m, p, i, j] = WS * W1[128*(2p+i)+k, 128m+j]
    w1_dr = np.ascontiguousarray(
        (pd_w1 * WS).reshape(4, 2, 128, 4, 128).transpose(2, 3, 0, 1, 4)
    ).astype(F8)
    # dcat[h, a*8+d] = down_w[a, h, d]
    dcat = down_w.transpose(1, 0, 2).reshape(H, 32) * WS
    dw_dr = np.ascontiguousarray(
        dcat.reshape(4, 2, 128, 32).transpose(2, 0, 1, 3)
    ).astype(F8)
    # w2 (x16, fp8 DR layout) replicated across 32 cols:
    # w2rep[k, p, i, j] = 16 * w2[128*(2p+i)+k]
    w2rep = np.ascontiguousarray(
        np.repeat((16.0 * pd_w2).reshape(2, 2, 128).transpose(2, 0, 1)[:, :, :, None], 32, axis=3)
    ).astype(F8)
    iim = np.zeros((128, 2, 128), f32)
    iim[np.arange(128), 0, np.arange(128)] = US
    iim[np.arange(128), 1, np.arange(128)] = US
    iim = iim.astype(F8)
    u36m = np.zeros((32, 2, H), f32)
    u36m[:, 0, :] = SCALE * US * up_w.reshape(32, H)
    u36m[0:4, 1, :] = SCALE * US * up_b
    u36m = u36m.astype(f16)

    cfm = np.zeros((128, _CF_COLS), f32)
    cfm[:, 0:4] = pd_b1.reshape(4, 128).T
    cfm[0:32, 4] = down_b.reshape(32)
    cfm[:, 5] = 16.0 * (math.log(THRESH / (1.0 - THRESH)) - float(pd_b2[0]))
    cfm[0:4, 6] = sel_b / 2.0
    cfm[0, 7:11] = 1.0
    cfm[0:4, 11:15] = np.eye(4)
    for r in range(32):
        cfm[r // 8, 15 + r] = 1.0
    cfm[0:4, 47:51] = np.eye(4)

    selw = np.ascontiguousarray(
        sel_w.reshape(8, 128, 4).transpose(1, 0, 2).reshape(128, 32)
    ).astype(f16)
    return dict(w1=w1_dr, dw=dw_dr, w2r=w2rep, ii=iim, u36=u36m, cf=cfm), selw


def _host_core(xb, selw):
    """Per-core inputs from one batch row xb [S, H] f32."""
    xt = np.ascontiguousarray(xb.T)                     # [H, S]
    hi = xt.astype(F8)
    lo = (xt - hi.astype(np.float32)).astype(F8)
    x8m = np.stack([hi, lo], 0)                         # [s, H, S]
    x8m = x8m.reshape(2, 8, 128, NT, T)                 # [s, c, p, i, t]
    x8m = np.ascontiguousarray(x8m.transpose(3, 2, 1, 0, 4))  # [i, p, c, s, t]
    csm = np.zeros((128, _CS_COLS), np.float16)
    csm[:, 0:32] = selw
    csm[:, 32:40] = xb[-1].reshape(8, 128).T.astype(np.float16)
    return dict(x8=x8m, cs=csm)


def _run(inputs, trace=False, **kwargs):
    nc = _get_nc()
    shared, selw = _host_shared(inputs)
    hs = np.asarray(inputs["hidden_states"], np.float32)
    in_maps = [dict(shared, **_host_core(hs[b], selw)) for b in range(N_CORES)]
    try:
        res = run_bass_kernel_spmd(
            nc, in_maps, core_ids=list(range(N_CORES)), trace=trace, **kwargs
        )
    except ModuleNotFoundError:
        res = run_bass_kernel_spmd(
            nc, in_maps, core_ids=list(range(N_CORES)), trace=False, **kwargs
        )
    outs = []
    for b in range(N_CORES):
        ob = np.asarray(res.results[b]["out"])          # [NT, 128, 8, T] f16
        outs.append(ob.transpose(0, 3, 2, 1).reshape(S, H).astype(np.float32))
    return np.stack(outs, 0), res


def kernel(**inputs) -> np.ndarray:
    out, _ = _run(inputs, trace=False)
    return out


# revision 27
# speedup vs baseline: 2.0887x; 1.0188x over previous
"""Trainium2 Bass kernel for nn_CRFTModule (moe_routing).

Pure data parallel over batch: 8 cores, one batch row (4096 tokens) each.

Math per core (batch b, S=4096 tokens, H=1024):
  z      = gelu(x @ W1 + b1) @ W2                    (critical-path detector)
  mask   = z > logit(0.7) - b2                       (compare in logit space)
  aw     = softmax(x[last] @ sel_w + sel_b)          (adapter selector, 4-way)
  t      = gelu(x @ Dcat + db)                       (all 4 down-projs, [S,32])
  out    = x + mask * (sum_a 0.3*aw[a] (t_a @ up_w[a] + up_b[a]))

Layout strategy (everything transposed, fp8-heavy):
  - Host pre-transposes x to x^T [H, S] and ships it as an fp8 e4m3 hi/lo
    pair (hi = fp8(x), lo = fp8(x - hi); hi+lo reconstructs x to ~7e-4 rel,
    same byte count as f16).  Tiles of 512 tokens: [128, 8 chunks, 2, 512].
  - Detector mm1 and the down-proj run as fp8 DoubleRow matmuls (K=256 per
    pass) on the hi planes; W1/Dcat are pre-scaled by 8 on the host with
    1/8 folded into the gelu activation's scale argument.
  - mm2 (z = h @ W2) stays f16 with W2 replicated across 32 columns, so the
    psum is the z row broadcast to 32 partitions; the mask is taken with a
    DVE is_gt and folded into G (gelu(down) + ones rows) BEFORE the up-proj.
  - The residual add runs on the PE: each up-proj psum group starts with a
    16*I DoubleRow identity matmul that sums 16*(hi+lo) into psum; the
    up weights are pre-scaled by 0.3*16, so the drain is a pure psum*(1/16)
    copy to f16, split across ACT/DVE/GPSIMD.
  - Output is written transposed f16 [8, 128, 8, 512]; the host transposes
    back and upcasts.
"""
import math

import numpy as np
import ml_dtypes

import concourse.bacc as bacc
import concourse.mybir as mybir
from concourse.tile import TileContext
from concourse.bass_utils import run_bass_kernel_spmd

dt = mybir.dt
AF = mybir.ActivationFunctionType
ALU = mybir.AluOpType
DR = mybir.MatmulPerfMode.DoubleRow

B, S, H = 8, 4096, 1024
A_DIM, N_ADAPT = 8, 4
PD = H // 2
T = 512
NT = S // T
N_CORES = 8
THRESH, SCALE = 0.7, 0.3
WS = 8.0      # host prescale on W1/Dcat (fp8 range); 1/WS folded into gelu scale
US = 16.0     # host prescale on up weights + residual identity; drain scales 1/US

F8 = ml_dtypes.float8_e4m3

# f32 const blob columns: b1(0:4) db(4:5) thr(5:6) selb2(6:7) o14(7:11)
# i4(11:15) e32(15:47) e32b(47:79)
_CF_COLS = 79
# f16 const blob columns: selw(0:32) xlast(32:40)
_CS_COLS = 40

# drain engine per H-chunk (GPSIMD cannot read PSUM): ACT has the gelus too
_DRAIN_ENG = ["act", "act", "act", "dve", "dve", "dve", "dve", "dve"]


def _build():
    nc = bacc.Bacc("TRN2", target_bir_lowering=False, debug=False)

    x8 = nc.declare_dram_parameter("x8", [NT, 128, 8, 2, T], dt.float8e4, isOutput=False)
    w1 = nc.declare_dram_parameter("w1", [128, 4, 4, 2, 128], dt.float8e4, isOutput=False)
    dw = nc.declare_dram_parameter("dw", [128, 4, 2, 32], dt.float8e4, isOutput=False)
    ii = nc.declare_dram_parameter("ii", [128, 2, 128], dt.float8e4, isOutput=False)
    w2r = nc.declare_dram_parameter("w2r", [128, 2, 2, 32], dt.float8e4, isOutput=False)
    u36 = nc.declare_dram_parameter("u36", [32, 2, H], dt.float16, isOutput=False)
    cf = nc.declare_dram_parameter("cf", [128, _CF_COLS], dt.float32, isOutput=False)
    cs = nc.declare_dram_parameter("cs", [128, _CS_COLS], dt.float16, isOutput=False)
    out = nc.declare_dram_parameter("out", [NT, 128, 8, T], dt.float16, isOutput=True)

    with TileContext(nc) as tc:
        with (
            tc.tile_pool(name="consts", bufs=1) as cp,
            tc.tile_pool(name="work", bufs=2) as wp,
            tc.tile_pool(name="psum", bufs=2, space="PSUM") as pp,
        ):
            # ---- constant loads: mm1's operands (w1, x8[0]) go FIRST; each
            # small DMA costs ~650ns of SP sequencer issue time, so the small
            # blobs follow the two transfers that gate the first matmul ----
            w1_sb = cp.tile([128, 4, 4, 2, 128], dt.float8e4, tag="w1")
            nc.sync.dma_start(out=w1_sb[:, 0, :, :, :], in_=w1[:, 0, :, :, :])
            xt0 = wp.tile([128, 8, 2, T], dt.float8e4, tag="x8", name="x0", bufs=8)
            nc.sync.dma_start(out=xt0[:], in_=x8[0])
            nc.sync.dma_start(out=w1_sb[:, 1:4, :, :, :], in_=w1[:, 1:4, :, :, :])
            cs_sb = cp.tile([128, _CS_COLS], dt.float16, tag="cs")
            nc.sync.dma_start(out=cs_sb[:], in_=cs[:])
            cf_sb = cp.tile([128, _CF_COLS], dt.float32, tag="cf")
            nc.sync.dma_start(out=cf_sb[:], in_=cf[:])
            w2_sb = cp.tile([128, 2, 2, 32], dt.float8e4, tag="w2")
            nc.sync.dma_start(out=w2_sb[:], in_=w2r[:])
            dw_sb = cp.tile([128, 4, 2, 32], dt.float8e4, tag="dw")
            nc.sync.dma_start(out=dw_sb[:], in_=dw[:])
            ii_sb = cp.tile([128, 2, 128], dt.float8e4, tag="ii")
            nc.sync.dma_start(out=ii_sb[:], in_=ii[:])
            u36_sb = cp.tile([32, 2, H], dt.float16, tag="u36")
            nc.sync.dma_start(out=u36_sb[:], in_=u36[:])

            b1v = cf_sb[:, 0:4]
            dbv = cf_sb[0:32, 4:5]
            thrv = cf_sb[0:32, 5:6]
            selb2 = cf_sb[0:4, 6:7]
            o14v = cf_sb[0:1, 7:11]
            i4v = cf_sb[0:4, 11:15]
            e32v = cf_sb[0:4, 15:47]
            e32bv = cf_sb[0:4, 47:79]

            # dummy ACT op so the gelu/tanh table set is resident early
            dummy = cp.tile([1, 1], dt.float16, tag="dummy")
            nc.scalar.copy(dummy[:], cs_sb[0:1, 0:1])

            # ---- adapter selector (once per core) -> fold into up weights ----
            # selector psums borrow the "hps" tag (bank-granular allocator;
            # they rotate through the same 2 banks before the main loop)
            ps_sel = pp.tile([128, T], dt.float32, tag="hps", name="ps_sel", bufs=2)[0:4, 0:1]
            for c in range(8):
                nc.tensor.matmul(
                    ps_sel, cs_sb[:, 4 * c : 4 * c + 4], cs_sb[:, 32 + c : 33 + c],
                    start=(c == 0), stop=(c == 7),
                )
            # softmax via tanh identity: exp(z+b) = (1+t)/(1-t), t=tanh((z+b)/2)
            t4 = cp.tile([4, 1], dt.float32, tag="t4")
            nc.scalar.activation(t4[:], ps_sel, AF.Tanh, bias=selb2, scale=0.5)
            num4 = cp.tile([4, 1], dt.float32, tag="num4")
            nc.vector.tensor_scalar(num4[:], t4[:], 1.0, None, ALU.add)
            den4 = cp.tile([4, 1], dt.float32, tag="den4")
            nc.vector.tensor_scalar(den4[:], t4[:], -1.0, 1.0, ALU.mult, ALU.add)
            rden4 = cp.tile([4, 1], dt.float32, tag="rden4")
            nc.vector.reciprocal(rden4[:], den4[:])
            e4 = cp.tile([4, 1], dt.float32, tag="e4")
            nc.vector.tensor_tensor(e4[:], num4[:], rden4[:], ALU.mult)
            ps_et = pp.tile([128, T], dt.float32, tag="hps", name="ps_et", bufs=2)[0:1, 0:4]
            nc.tensor.matmul(ps_et, e4[:], i4v, start=True, stop=True)
            ssum = cp.tile([1, 1], dt.float32, tag="ssum")
            nc.vector.reduce_sum(ssum[:], ps_et, axis=mybir.AxisListType.X)
            rsum = cp.tile([1, 1], dt.float32, tag="rsum")
            nc.vector.reciprocal(rsum[:], ssum[:])
            ps_rs = pp.tile([128, T], dt.float32, tag="hps", name="ps_rs", bufs=2)[0:4, 0:1]
            nc.tensor.matmul(ps_rs, o14v, rsum[:], start=True, stop=True)
            w4 = cp.tile([4, 1], dt.float32, tag="w4")
            nc.vector.tensor_tensor(w4[:], e4[:], ps_rs, ALU.mult)
            # wv32 = per-row adapter weight for up_w rows; wvB for up_b rows
            ps_w32 = pp.tile([128, T], dt.float32, tag="hps", name="ps_w32", bufs=2)[0:32, 0:1]
            nc.tensor.matmul(ps_w32, e32v, w4[:], start=True, stop=True)
            ps_wB = pp.tile([128, T], dt.float32, tag="hps", name="ps_wB", bufs=2)[0:32, 0:1]
            nc.tensor.matmul(ps_wB, e32bv, w4[:], start=True, stop=True)
            wv32 = cp.tile([32, 1], dt.float32, tag="wv32")
            nc.vector.tensor_copy(wv32[:], ps_w32)
            wvB = cp.tile([32, 1], dt.float32, tag="wvB")
            nc.vector.tensor_copy(wvB[:], ps_wB)
            uw8 = cp.tile([32, 2, H], dt.float8e4, tag="uw8")
            nc.vector.tensor_scalar(uw8[:, 0, :], u36_sb[:, 0, :], wv32[:], None, ALU.mult)
            nc.vector.tensor_scalar(uw8[:, 1, :], u36_sb[:, 1, :], wvB[:], None, ALU.mult)

            # masked-G double buffers; the zero rows are written once
            gms = [
                cp.tile([32, 2, T], dt.float8e4, tag=f"gm{k}", name=f"gm{k}")
                for k in range(3)
            ]
            for k in range(3):
                nc.gpsimd.memset(gms[k][0:32, 1, :], 0.0)

            # ---- software-pipelined main loop (3 stages deep) ----
            # PE emission interleaves mm1 chunks of tile i with up-proj pairs
            # of tile i-2 so the gelu chain on ACT starts as early as possible
            hist = {}
            for i in range(NT + 2):
                xt = hsb = None
                if i < NT:
                    if i == 0:
                        xt = xt0
                    else:
                        xt = wp.tile([128, 8, 2, T], dt.float8e4, tag="x8", name=f"x{i}", bufs=8)
                        nc.sync.dma_start(out=xt[:], in_=x8[i])
                    hsb = wp.tile([128, 4, T], dt.float8e4, tag="h", name=f"h{i}", bufs=2)

                bprev = None
                if i - 2 >= 0:
                    bprev = hist.pop(i - 2)
                    osb = wp.tile([128, 8, T], dt.float16, tag="o", name=f"o{i-2}", bufs=4)

                def mm1_chunk(m):
                    psh = pp.tile([128, T], dt.float32, tag="hps", name=f"hps{i}_{m}", bufs=2)
                    for p in range(4):
                        nc.tensor.matmul(
                            psh[:], w1_sb[:, m, p, :, :], xt[:, 2 * p : 2 * p + 2, 0, :],
                            start=(p == 0), stop=(p == 3), perf_mode=DR,
                        )
                    nc.scalar.activation(
                        hsb[:, m, :], psh[:], AF.Gelu,
                        bias=b1v[:, m : m + 1], scale=1.0 / WS,
                    )

                def up_pair(q):
                    xtp, gmp = bprev
                    j = i - 2
                    psu = pp.tile([128, 2, T], dt.float32, tag="ups", name=f"ups{j}_{q}", bufs=2)
                    for d in range(2):
                        c = 2 * q + d
                        # residual: 16*(hi+lo) via DoubleRow identity
                        nc.tensor.matmul(
                            psu[:, d, :], ii_sb[:], xtp[:, c, :, :],
                            start=True, stop=False, perf_mode=DR,
                        )
                        # up-proj, mask+softmax+0.3*16 already folded in
                        nc.tensor.matmul(
                            psu[:, d, :], uw8[:, :, 128 * c : 128 * (c + 1)], gmp[:],
                            start=False, stop=True, perf_mode=DR,
                        )
                    j_last = j >= NT - 2
                    if q == 0 or (j_last and q == 2):
                        nc.scalar.mul(osb[:, 2 * q : 2 * q + 2, :], psu[:], 1.0 / US)
                    else:
                        nc.vector.tensor_scalar(
                            osb[:, 2 * q : 2 * q + 2, :], psu[:], 1.0 / US, None, ALU.mult
                        )

                # interleave: m0 | q0 m1 | q1 m2 | q2 m3 | q3
                if xt is not None:
                    mm1_chunk(0)
                for q in range(4):
                    if bprev is not None:
                        up_pair(q)
                    if xt is not None and q < 3:
                        mm1_chunk(q + 1)

                if bprev is not None:
                    j = i - 2
                    # SWDGE path: a waiting store must not block later loads
                    # queued behind it on the SP sequencer; the last store goes
                    # in halves so its first half overlaps the final drains
                    if j == NT - 1:
                        for qq in range(4):
                            nc.gpsimd.dma_start(
                                out=out[j, :, 2 * qq : 2 * qq + 2, :],
                                in_=osb[:, 2 * qq : 2 * qq + 2, :],
                            )
                    elif j == NT - 2:
                        nc.gpsimd.dma_start(out=out[j, :, 0:4, :], in_=osb[:, 0:4, :])
                        nc.gpsimd.dma_start(out=out[j, :, 4:8, :], in_=osb[:, 4:8, :])
                    else:
                        nc.gpsimd.dma_start(out=out[j], in_=osb[:])

                if xt is not None:
                    # down-proj (fp8 DoubleRow) + gelu
                    pst = pp.tile([32, T], dt.float32, tag="tps", name=f"tps{i}", bufs=1)
                    for p in range(4):
                        nc.tensor.matmul(
                            pst[:], dw_sb[:, p, :, :], xt[:, 2 * p : 2 * p + 2, 0, :],
                            start=(p == 0), stop=(p == 3), perf_mode=DR,
                        )
                    g0 = wp.tile([32, T], dt.float16, tag="g0", name=f"g0{i}", bufs=2)
                    nc.scalar.activation(g0[:], pst[:], AF.Gelu, bias=dbv, scale=1.0 / WS)

                    # detector mm2 (fp8 DoubleRow, W2*16 replicated to 32 cols);
                    # psum holds 16*z so the threshold is prescaled by 16 too
                    psz = pp.tile([32, T], dt.float32, tag="zps", name=f"zps{i}", bufs=1)
                    for p in range(2):
                        nc.tensor.matmul(
                            psz[:], w2_sb[:, p, :, :], hsb[:, 2 * p : 2 * p + 2, :],
                            start=(p == 0), stop=(p == 1), perf_mode=DR,
                        )
                    msk = wp.tile([32, T], dt.float16, tag="msk", name=f"msk{i}", bufs=2)
                    nc.vector.tensor_scalar(msk[:], psz[:], thrv, None, ALU.is_gt)
                    gm = gms[i % 3]
                    nc.gpsimd.tensor_tensor(gm[:, 0, :], g0[:], msk[:], ALU.mult)
                    nc.gpsimd.tensor_copy(gm[0:4, 1, :], msk[0:4, :])
                    hist[i] = (xt, gm)

    nc.compile()
    return nc


_CACHE = {}


def _get_nc():
    if "nc" not in _CACHE:
        _CACHE["nc"] = _build()
    return _CACHE["nc"]


def _host_shared(inputs):
    f32, f16 = np.float32, np.float16
    pd_w1 = np.asarray(inputs["pd_w1"], f32)
    pd_b1 = np.asarray(inputs["pd_b1"], f32)
    pd_w2 = np.asarray(inputs["pd_w2"], f32)
    pd_b2 = np.asarray(inputs["pd_b2"], f32)
    down_w = np.asarray(inputs["down_w"], f32)
    down_b = np.asarray(inputs["down_b"], f32)
    up_w = np.asarray(inputs["up_w"], f32)
    up_b = np.asarray(inputs["up_b"], f32)
    sel_w = np.asarray(inputs["sel_w"], f32)
    sel_b = np.asarray(inputs["sel_b"], f32)

    # w1_dr[k, m, p, i, j] = WS * W1[128*(2p+i)+k, 128m+j]
    w1_dr = np.ascontiguousarray(
        (pd_w1 * WS).reshape(4, 2, 128, 4, 128).transpose(2, 3, 0, 1, 4)
    ).astype(F8)
    # dcat[h, a*8+d] = down_w[a, h, d]
    dcat = down_w.transpose(1, 0, 2).reshape(H, 32) * WS
    dw_dr = np.ascontiguousarray(
        dcat.reshape(4, 2, 128, 32).transpose(2, 0, 1, 3)
    ).astype(F8)
    # w2 (x16, fp8 DR layout) replicated across 32 cols:
    # w2rep[k, p, i, j] = 16 * w2[128*(2p+i)+k]
    w2rep = np.ascontiguousarray(
        np.repeat((16.0 * pd_w2).reshape(2, 2, 128).transpose(2, 0, 1)[:, :, :, None], 32, axis=3)
    ).astype(F8)
    iim = np.zeros((128, 2, 128), f32)
    iim[np.arange(128), 0, np.arange(128)] = US
    iim[np.arange(128), 1, np.arange(128)] = US
    iim = iim.astype(F8)
    u36m = np.zeros((32, 2, H), f32)
    u36m[:, 0, :] = SCALE * US * up_w.reshape(32, H)
    u36m[0:4, 1, :] = SCALE * US * up_b
    u36m = u36m.astype(f16)

    cfm = np.zeros((128, _CF_COLS), f32)
    cfm[:, 0:4] = pd_b1.reshape(4, 128).T
    cfm[0:32, 4] = down_b.reshape(32)
    cfm[:, 5] = 16.0 * (math.log(THRESH / (1.0 - THRESH)) - float(pd_b2[0]))
    cfm[0:4, 6] = sel_b / 2.0
    cfm[0, 7:11] = 1.0
    cfm[0:4, 11:15] = np.eye(4)
    for r in range(32):
        cfm[r // 8, 15 + r] = 1.0
    cfm[0:4, 47:51] = np.eye(4)

    selw = np.ascontiguousarray(
        sel_w.reshape(8, 128, 4).transpose(1, 0, 2).reshape(128, 32)
    ).astype(f16)
    return dict(w1=w1_dr, dw=dw_dr, w2r=w2rep, ii=iim, u36=u36m, cf=cfm), selw


def _host_core(xb, selw):
    """Per-core inputs from one batch row xb [S, H] f32."""
    xt = np.ascontiguousarray(xb.T)                     # [H, S]
    hi = xt.astype(F8)
    lo = (xt - hi.astype(np.float32)).astype(F8)
    x8m = np.stack([hi, lo], 0)                         # [s, H, S]
    x8m = x8m.reshape(2, 8, 128, NT, T)                 # [s, c, p, i, t]
    x8m = np.ascontiguousarray(x8m.transpose(3, 2, 1, 0, 4))  # [i, p, c, s, t]
    csm = np.zeros((128, _CS_COLS), np.float16)
    csm[:, 0:32] = selw
    csm[:, 32:40] = xb[-1].reshape(8, 128).T.astype(np.float16)
    return dict(x8=x8m, cs=csm)


def _run(inputs, trace=False, **kwargs):
    nc = _get_nc()
    shared, selw = _host_shared(inputs)
    hs = np.asarray(inputs["hidden_states"], np.float32)
    in_maps = [dict(shared, **_host_core(hs[b], selw)) for b in range(N_CORES)]
    try:
        res = run_bass_kernel_spmd(
            nc, in_maps, core_ids=list(range(N_CORES)), trace=trace, **kwargs
        )
    except ModuleNotFoundError:
        res = run_bass_kernel_spmd(
            nc, in_maps, core_ids=list(range(N_CORES)), trace=False, **kwargs
        )
    outs = []
    for b in range(N_CORES):
        ob = np.asarray(res.results[b]["out"])          # [NT, 128, 8, T] f16
        outs.append(ob.transpose(0, 3, 2, 1).reshape(S, H).astype(np.float32))
    return np.stack(outs, 0), res


def kernel(**inputs) -> np.ndarray:
    out, _ = _run(inputs, trace=False)
    return out
